# revision 12
# baseline (speedup 1.0000x reference)
"""Trainium2 distributed kernel for nn_AMI_38130719653974 (retrieval_knn).

posterior[q, c] = (w_r*exp(-d_rgb) + w_f*exp(-d_flow)) / rowsum(...)
  d_F = ||(tgt_F[q] + eps) - ctx_F[c]||_2,  w_r = c_r/(c_r+c_f), w_f = 1-w_r

Sharding: queries split across 8 NeuronCores (512 rows each); context
features replicated. Each core normalizes its own rows — no collectives.

Per-core device algorithm:
  - GEMM (bf16): PSUM[q, c] = ones_f16.T @ ||b||^2_row + sum_k (-2*aT_k).T @ b_k
    (the f16 "inject" matmul adds the per-context squared norm into the
    same PSUM accumulation group, so PSUM = ||b||^2 - 2 a.b directly;
    f16 keeps the inject a single HW pass — f32 matmuls lower to 2 —
    with ~0.5 abs error on ||b||^2 ~ 1024, i.e. <0.6% on exp(-d))
  - ACT: d = Sqrt(PSUM + bias=||a||^2)   (read straight from PSUM)
         e = Exp(-d) in place, accum_out gives row sums for free
  - DVE: w_r/w_f weighting + reciprocal + normalize, in place
  - ||b||^2 via ones-vector matmul over b^2, accumulated in per-1024-col
    PSUM row chunks so the main GEMM's injects unblock as ctx streams in;
    ||a||^2 via ACT Square with accum_out on row-major target shards.
  - f32->bf16 context casts run on the otherwise idle GpSimd engine.

eps=1e-6 from the reference shifts d by <1e-6 relative — far below bf16
GEMM noise — and is deliberately ignored.

Host side only reshapes/shards numpy arrays (transposed copies for DMA
layout); all arithmetic happens on device.
"""
import sys

sys.path.insert(0, '/opt/trn_rl_repo')

import numpy as np
import concourse.bass as bass
import concourse.bacc as bacc
import concourse.mybir as mybir
from concourse.tile import TileContext
from concourse.tile_rust import add_dep_helper
from concourse.bass_utils import run_bass_kernel_spmd

F32 = mybir.dt.float32
F16 = mybir.dt.float16
BF16 = mybir.dt.bfloat16
AF = mybir.ActivationFunctionType
MUL = mybir.AluOpType.mult
ADD = mybir.AluOpType.add

P = 128
Q = 4096
C = 4096
D = 1024
NCORES = 8
QL = Q // NCORES          # 512 queries per core
MT = QL // P              # 4 m-tiles
KT = D // P               # 8 k-tiles
NW = 512                  # n-tile width (one PSUM bank)
NT = C // NW              # 8 n-tiles
CCH = 1024                # ctx streaming chunk width
FEATS = ("rgb", "flow")


def build_kernel():
    nc = bacc.Bacc(None, target_bir_lowering=False)

    ctxT = {f: nc.dram_tensor(f"ctxT_{f}", [D, C], F32, kind="ExternalInput")
            for f in FEATS}
    tgtT = {f: nc.dram_tensor(f"tgtT_{f}", [D, QL], F32, kind="ExternalInput")
            for f in FEATS}
    tgtR = {f: nc.dram_tensor(f"tgtR_{f}", [QL, D], F32, kind="ExternalInput")
            for f in FEATS}
    crw_d = nc.dram_tensor("crw", [MT, P], F32, kind="ExternalInput")
    cfw_d = nc.dram_tensor("cfw", [MT, P], F32, kind="ExternalInput")
    out_d = nc.dram_tensor("out", [QL, C], F32, kind="ExternalOutput")

    with TileContext(nc) as tc:
        with tc.tile_pool(name="ctx", bufs=1) as ctx_pool, \
             tc.tile_pool(name="stg", bufs=4) as stg_pool, \
             tc.tile_pool(name="bsq", bufs=2) as bsq_pool, \
             tc.tile_pool(name="a2", bufs=1) as a2_pool, \
             tc.tile_pool(name="dd", bufs=2) as d_pool, \
             tc.tile_pool(name="small", bufs=1) as small, \
             tc.tile_pool(name="srows", bufs=2) as srow_pool:

            # ---- constants / weights --------------------------------------
            ones1 = small.tile([33, P], F16, tag="ones1")
            nc.vector.memset(ones1[:], 1.0)
            onesk = small.tile([P, 1], BF16, tag="onesk")
            nc.vector.memset(onesk[:], 1.0)

            crt = small.tile([P, MT], F32, tag="crt")
            cft = small.tile([P, MT], F32, tag="cft")
            nc.sync.dma_start(crt[:], crw_d.ap().rearrange("m p -> p m"))
            nc.sync.dma_start(cft[:], cfw_d.ap().rearrange("m p -> p m"))
            denom = small.tile([P, MT], F32, tag="denom")
            nc.vector.tensor_tensor(denom[:], crt[:], cft[:], op=ADD)
            rden = small.tile([P, MT], F32, tag="rden")
            nc.vector.reciprocal(rden[:], denom[:])
            wr = small.tile([P, MT], F32, tag="wr")
            wf = small.tile([P, MT], F32, tag="wf")
            nc.vector.tensor_tensor(wr[:], crt[:], rden[:], op=MUL)
            nc.vector.tensor_tensor(wf[:], cft[:], rden[:], op=MUL)

            # ---- ||a||^2 per query (ACT Square + accum_out) ----------------
            sqa = {}
            for f in FEATS:
                sqa[f] = small.tile([P, MT], F32, tag=f"sqa_{f}",
                                    name=f"sqa_{f}")
                for m in range(MT):
                    st = stg_pool.tile([P, D], F32, tag="stg")
                    nc.sync.dma_start(st[:], tgtR[f][m * P:(m + 1) * P, :])
                    nc.scalar.activation(st[:], st[:], AF.Square,
                                         accum_out=sqa[f][:, m:m + 1])

            # ---- a2 = -2*targets (bf16, lhsT layout), all m-tiles ----------
            a2 = {}
            for m in range(MT):
                for f in FEATS:
                    a2s = stg_pool.tile([P, D], F32, tag="stg", name="a2s")
                    nc.sync.dma_start(
                        a2s[:].rearrange("p (k q) -> p k q", k=KT),
                        tgtT[f][:, m * P:(m + 1) * P].rearrange(
                            "(k p) q -> p k q", p=P))
                    a2[m, f] = a2_pool.tile([P, D], BF16,
                                            tag=f"a2_{m}_{f}",
                                            name=f"a2_{m}_{f}")
                    nc.gpsimd.tensor_scalar_mul(a2[m, f][:], a2s[:], -2.0)

            # ---- ctx stream: f32->bf16 (GpSimd) + ||b||^2 rows (PE) --------
            # h-outer so the main GEMM unblocks chunk by chunk
            b = {f: [ctx_pool.tile([P, C], BF16, tag=f"b_{f}_{k}",
                                   name=f"b_{f}_{k}")
                     for k in range(KT)] for f in FEATS}
            sqb2 = small.tile([33, C], F16, tag="sqb2")
            with tc.tile_pool(name="psrow", bufs=4, space="PSUM") as psrow_pool:
                for h in range(C // CCH):
                    for fi, f in enumerate(FEATS):
                        prow = psrow_pool.tile([1, CCH], F32, tag="prow",
                                               name=f"prow_{f}_{h}")
                        for k in range(KT):
                            st = stg_pool.tile([P, CCH], F32, tag="stg")
                            nc.sync.dma_start(
                                st[:], ctxT[f][k * P:(k + 1) * P,
                                               h * CCH:(h + 1) * CCH])
                            nc.gpsimd.tensor_copy(
                                b[f][k][:, h * CCH:(h + 1) * CCH], st[:])
                            bsq = bsq_pool.tile([P, CCH], BF16, tag="bsq")
                            nc.vector.tensor_tensor(bsq[:], st[:], st[:],
                                                    op=MUL)
                            for j in range(CCH // NW):
                                nc.tensor.matmul(
                                    prow[:, j * NW:(j + 1) * NW], onesk[:],
                                    bsq[:, j * NW:(j + 1) * NW],
                                    start=(k == 0), stop=(k == KT - 1),
                                    skip_group_check=True)
                        nc.scalar.copy(
                            sqb2[fi * 32:fi * 32 + 1, h * CCH:(h + 1) * CCH],
                            prow[:])

            ps_cm = tc.tile_pool(name="ps", bufs=8, space="PSUM")
            ps_pool = ps_cm.__enter__()

            # ---- main loop over m-tiles ------------------------------------
            prev_exps = []
            for m in range(MT):
                d = {}
                sqrt_insts = []
                for fi, f in enumerate(FEATS):
                    d[f] = d_pool.tile([P, C], F32, tag="d", name=f"d_{f}")
                    for n in range(NT):
                        ps = ps_pool.tile([P, NW], F32, tag="ps")
                        nc.tensor.matmul(
                            ps[:], ones1[fi * 32:fi * 32 + 1, :],
                            sqb2[fi * 32:fi * 32 + 1, n * NW:(n + 1) * NW],
                            start=True, stop=False, skip_group_check=True)
                        for k in range(KT):
                            nc.tensor.matmul(
                                ps[:], a2[m, f][:, k * P:(k + 1) * P],
                                b[f][k][:, n * NW:(n + 1) * NW],
                                start=False, stop=(k == KT - 1),
                                skip_group_check=True)
                        si = nc.scalar.activation(d[f][:, n * NW:(n + 1) * NW],
                                                  ps[:], AF.Sqrt,
                                                  bias=sqa[f][:, m:m + 1])
                        sqrt_insts.append(si)
                        # keep ACT table order: sqrt(m) after exp(m-1)
                        for pe in prev_exps:
                            add_dep_helper(si.ins, pe.ins, sync=False,
                                           reason="act table order")

                srow = {}
                exps = []
                for f in FEATS:
                    srow[f] = srow_pool.tile([P, 1], F32, tag="srow",
                                             name=f"srow_{f}")
                    ei = nc.scalar.activation(d[f][:], d[f][:], AF.Exp,
                                              scale=-1.0, accum_out=srow[f][:])
                    # exp only after every sqrt of this m-tile (both feats)
                    for si in sqrt_insts:
                        add_dep_helper(ei.ins, si.ins, sync=False,
                                       reason="act table order")
                    exps.append(ei)
                prev_exps = exps

                # s = wr*srow_r + wf*srow_f ; u_F = w_F / s
                t1 = small.tile([P, 1], F32, tag="t1")
                nc.vector.tensor_tensor(t1[:], wr[:, m:m + 1], srow["rgb"][:],
                                        op=MUL)
                nc.vector.scalar_tensor_tensor(t1[:], srow["flow"][:],
                                               wf[:, m:m + 1], t1[:],
                                               op0=MUL, op1=ADD)
                rs = small.tile([P, 1], F32, tag="rs")
                nc.vector.reciprocal(rs[:], t1[:])
                ur = small.tile([P, 1], F32, tag="ur")
                uf = small.tile([P, 1], F32, tag="uf")
                nc.vector.tensor_tensor(ur[:], wr[:, m:m + 1], rs[:], op=MUL)
                nc.vector.tensor_tensor(uf[:], wf[:, m:m + 1], rs[:], op=MUL)

                # out = e_rgb*ur + e_flow*uf  (2 full-width DVE passes)
                nc.vector.tensor_scalar_mul(d["rgb"][:], d["rgb"][:], ur[:])
                nc.vector.scalar_tensor_tensor(d["rgb"][:], d["flow"][:],
                                               uf[:], d["rgb"][:],
                                               op0=MUL, op1=ADD)
                nc.sync.dma_start(out_d[m * P:(m + 1) * P, :], d["rgb"][:])

            ps_cm.__exit__(None, None, None)

    nc.compile()
    return nc


_NC_CACHE = None


def kernel(context_rgb_features, context_flow_features,
           target_rgb_features, target_flow_features, c_r, c_f):
    global _NC_CACHE
    if _NC_CACHE is None:
        _NC_CACHE = build_kernel()
    nc = _NC_CACHE

    ctx_rgb = np.asarray(context_rgb_features, np.float32)
    ctx_flow = np.asarray(context_flow_features, np.float32)
    tgt_rgb = np.asarray(target_rgb_features, np.float32)
    tgt_flow = np.asarray(target_flow_features, np.float32)
    c_r = np.asarray(c_r, np.float32)
    c_f = np.asarray(c_f, np.float32)

    ctxT_rgb = np.ascontiguousarray(ctx_rgb.T)
    ctxT_flow = np.ascontiguousarray(ctx_flow.T)

    in_maps = []
    for i in range(NCORES):
        sl = slice(i * QL, (i + 1) * QL)
        in_maps.append({
            "ctxT_rgb": ctxT_rgb,
            "ctxT_flow": ctxT_flow,
            "tgtT_rgb": np.ascontiguousarray(tgt_rgb[sl].T),
            "tgtT_flow": np.ascontiguousarray(tgt_flow[sl].T),
            "tgtR_rgb": np.ascontiguousarray(tgt_rgb[sl]),
            "tgtR_flow": np.ascontiguousarray(tgt_flow[sl]),
            "crw": np.ascontiguousarray(c_r[sl].reshape(MT, P)),
            "cfw": np.ascontiguousarray(c_f[sl].reshape(MT, P)),
        })

    global _LAST_IN_MAPS
    _LAST_IN_MAPS = in_maps
    res = run_bass_kernel_spmd(nc, in_maps, core_ids=list(range(NCORES)))
    return np.concatenate([r["out"] for r in res.results], axis=0)


_LAST_IN_MAPS = None


# revision 14
# speedup vs baseline: 1.7741x; 1.7741x over previous
"""Trainium2 distributed kernel for nn_AMI_38130719653974 (retrieval_knn).

posterior[q, c] = (w_r*exp(-d_rgb) + w_f*exp(-d_flow)) / rowsum(...)
  d_F = ||(tgt_F[q] + eps) - ctx_F[c]||_2,  w_r = c_r/(c_r+c_f), w_f = 1-w_r

Sharding: queries split across 8 NeuronCores (512 rows each); context
features replicated. Each core normalizes its own rows — no collectives.

Per-core device algorithm:
  - GEMM (bf16): PSUM[q, c] = ones_f16.T @ ||b||^2_row + sum_k (-2*aT_k).T @ b_k
    (the f16 "inject" matmul adds the per-context squared norm into the
    same PSUM accumulation group, so PSUM = ||b||^2 - 2 a.b directly;
    f16 keeps the inject a single HW pass — f32 matmuls lower to 2 —
    with ~0.5 abs error on ||b||^2 ~ 1024, i.e. <0.6% on exp(-d))
  - ACT: d = Sqrt(PSUM + bias=||a||^2)   (read straight from PSUM)
         e = Exp(-d) in place, accum_out gives row sums for free
  - DVE: w_r/w_f weighting + reciprocal + normalize, in place
  - ||b||^2 via ones-vector matmul over b^2, accumulated in per-1024-col
    PSUM row chunks so the main GEMM's injects unblock as ctx streams in;
    ||a||^2 via ACT Square with accum_out on row-major target shards.
  - f32->bf16 context casts run on DVE (GpSimd measured 6x slower and
    its SBUF port-lock stalls DVE).

eps=1e-6 from the reference shifts d by <1e-6 relative — far below bf16
GEMM noise — and is deliberately ignored.

Host side only reshapes/shards numpy arrays (transposed copies for DMA
layout); all arithmetic happens on device.
"""
import sys

sys.path.insert(0, '/opt/trn_rl_repo')

import numpy as np
import concourse.bass as bass
import concourse.bacc as bacc
import concourse.mybir as mybir
from concourse.tile import TileContext
from concourse.tile_rust import add_dep_helper
from concourse.bass_utils import run_bass_kernel_spmd

F32 = mybir.dt.float32
F16 = mybir.dt.float16
BF16 = mybir.dt.bfloat16
AF = mybir.ActivationFunctionType
MUL = mybir.AluOpType.mult
ADD = mybir.AluOpType.add

P = 128
Q = 4096
C = 4096
D = 1024
NCORES = 8
QL = Q // NCORES          # 512 queries per core
MT = QL // P              # 4 m-tiles
KT = D // P               # 8 k-tiles
NW = 512                  # n-tile width (one PSUM bank)
NT = C // NW              # 8 n-tiles
CCH = 1024                # ctx streaming chunk width
FEATS = ("rgb", "flow")


def build_kernel():
    nc = bacc.Bacc(None, target_bir_lowering=False)

    ctxT = {f: nc.dram_tensor(f"ctxT_{f}", [D, C], F32, kind="ExternalInput")
            for f in FEATS}
    tgtT = {f: nc.dram_tensor(f"tgtT_{f}", [D, QL], F32, kind="ExternalInput")
            for f in FEATS}
    tgtR = {f: nc.dram_tensor(f"tgtR_{f}", [QL, D], F32, kind="ExternalInput")
            for f in FEATS}
    crw_d = nc.dram_tensor("crw", [MT, P], F32, kind="ExternalInput")
    cfw_d = nc.dram_tensor("cfw", [MT, P], F32, kind="ExternalInput")
    out_d = nc.dram_tensor("out", [QL, C], F32, kind="ExternalOutput")

    with TileContext(nc) as tc:
        with tc.tile_pool(name="ctx", bufs=1) as ctx_pool, \
             tc.tile_pool(name="stg", bufs=4) as stg_pool, \
             tc.tile_pool(name="bsq", bufs=2) as bsq_pool, \
             tc.tile_pool(name="a2", bufs=1) as a2_pool, \
             tc.tile_pool(name="dd", bufs=2) as d_pool, \
             tc.tile_pool(name="small", bufs=1) as small, \
             tc.tile_pool(name="srows", bufs=2) as srow_pool:

            # ---- constants / weights --------------------------------------
            ones1 = small.tile([33, P], F16, tag="ones1")
            nc.vector.memset(ones1[:], 1.0)
            onesk = small.tile([P, 1], BF16, tag="onesk")
            nc.vector.memset(onesk[:], 1.0)

            crt = small.tile([P, MT], F32, tag="crt")
            cft = small.tile([P, MT], F32, tag="cft")
            nc.sync.dma_start(crt[:], crw_d.ap().rearrange("m p -> p m"))
            nc.sync.dma_start(cft[:], cfw_d.ap().rearrange("m p -> p m"))
            denom = small.tile([P, MT], F32, tag="denom")
            nc.vector.tensor_tensor(denom[:], crt[:], cft[:], op=ADD)
            rden = small.tile([P, MT], F32, tag="rden")
            nc.vector.reciprocal(rden[:], denom[:])
            wr = small.tile([P, MT], F32, tag="wr")
            wf = small.tile([P, MT], F32, tag="wf")
            nc.vector.tensor_tensor(wr[:], crt[:], rden[:], op=MUL)
            nc.vector.tensor_tensor(wf[:], cft[:], rden[:], op=MUL)

            # ---- ||a||^2 per query (ACT Square + accum_out) ----------------
            sqa = {}
            for f in FEATS:
                sqa[f] = small.tile([P, MT], F32, tag=f"sqa_{f}",
                                    name=f"sqa_{f}")
                for m in range(MT):
                    st = stg_pool.tile([P, D], F32, tag="stg")
                    nc.sync.dma_start(st[:], tgtR[f][m * P:(m + 1) * P, :])
                    nc.scalar.activation(st[:], st[:], AF.Square,
                                         accum_out=sqa[f][:, m:m + 1])

            # ---- a2 = -2*targets (bf16, lhsT layout), all m-tiles ----------
            a2 = {}
            for m in range(MT):
                for f in FEATS:
                    a2s = stg_pool.tile([P, D], F32, tag="stg", name="a2s")
                    nc.sync.dma_start(
                        a2s[:].rearrange("p (k q) -> p k q", k=KT),
                        tgtT[f][:, m * P:(m + 1) * P].rearrange(
                            "(k p) q -> p k q", p=P))
                    a2[m, f] = a2_pool.tile([P, D], BF16,
                                            tag=f"a2_{m}_{f}",
                                            name=f"a2_{m}_{f}")
                    nc.vector.tensor_scalar_mul(a2[m, f][:], a2s[:], -2.0)

            # ---- ctx stream: f32->bf16 (GpSimd) + ||b||^2 rows (PE) --------
            # h-outer so the main GEMM unblocks chunk by chunk
            b = {f: [ctx_pool.tile([P, C], BF16, tag=f"b_{f}_{k}",
                                   name=f"b_{f}_{k}")
                     for k in range(KT)] for f in FEATS}
            sqb2 = small.tile([33, C], F16, tag="sqb2")
            with tc.tile_pool(name="psrow", bufs=4, space="PSUM") as psrow_pool:
                for h in range(C // CCH):
                    for fi, f in enumerate(FEATS):
                        prow = psrow_pool.tile([1, CCH], F32, tag="prow",
                                               name=f"prow_{f}_{h}")
                        for k in range(KT):
                            st = stg_pool.tile([P, CCH], F32, tag="stg")
                            nc.sync.dma_start(
                                st[:], ctxT[f][k * P:(k + 1) * P,
                                               h * CCH:(h + 1) * CCH])
                            bb = b[f][k][:, h * CCH:(h + 1) * CCH]
                            nc.vector.tensor_copy(bb, st[:])
                            bsq = bsq_pool.tile([P, CCH], BF16, tag="bsq")
                            nc.vector.tensor_tensor(bsq[:], bb, bb, op=MUL)
                            for j in range(CCH // NW):
                                nc.tensor.matmul(
                                    prow[:, j * NW:(j + 1) * NW], onesk[:],
                                    bsq[:, j * NW:(j + 1) * NW],
                                    start=(k == 0), stop=(k == KT - 1),
                                    skip_group_check=True)
                        nc.scalar.copy(
                            sqb2[fi * 32:fi * 32 + 1, h * CCH:(h + 1) * CCH],
                            prow[:])

            ps_cm = tc.tile_pool(name="ps", bufs=8, space="PSUM")
            ps_pool = ps_cm.__enter__()

            # ---- main loop over m-tiles ------------------------------------
            prev_exps = []
            for m in range(MT):
                d = {}
                sqrt_insts = []
                for fi, f in enumerate(FEATS):
                    d[f] = d_pool.tile([P, C], F32, tag="d", name=f"d_{f}")
                    for n in range(NT):
                        ps = ps_pool.tile([P, NW], F32, tag="ps")
                        nc.tensor.matmul(
                            ps[:], ones1[fi * 32:fi * 32 + 1, :],
                            sqb2[fi * 32:fi * 32 + 1, n * NW:(n + 1) * NW],
                            start=True, stop=False, skip_group_check=True)
                        for k in range(KT):
                            nc.tensor.matmul(
                                ps[:], a2[m, f][:, k * P:(k + 1) * P],
                                b[f][k][:, n * NW:(n + 1) * NW],
                                start=False, stop=(k == KT - 1),
                                skip_group_check=True)
                        si = nc.scalar.activation(d[f][:, n * NW:(n + 1) * NW],
                                                  ps[:], AF.Sqrt,
                                                  bias=sqa[f][:, m:m + 1])
                        sqrt_insts.append(si)
                        # keep ACT table order: sqrt(m) after exp(m-1)
                        for pe in prev_exps:
                            add_dep_helper(si.ins, pe.ins, sync=False,
                                           reason="act table order")

                srow = {}
                exps = []
                for f in FEATS:
                    srow[f] = srow_pool.tile([P, 1], F32, tag="srow",
                                             name=f"srow_{f}")
                    ei = nc.scalar.activation(d[f][:], d[f][:], AF.Exp,
                                              scale=-1.0, accum_out=srow[f][:])
                    # exp only after every sqrt of this m-tile (both feats)
                    for si in sqrt_insts:
                        add_dep_helper(ei.ins, si.ins, sync=False,
                                       reason="act table order")
                    exps.append(ei)
                prev_exps = exps

                # s = wr*srow_r + wf*srow_f ; u_F = w_F / s
                t1 = small.tile([P, 1], F32, tag="t1")
                nc.vector.tensor_tensor(t1[:], wr[:, m:m + 1], srow["rgb"][:],
                                        op=MUL)
                nc.vector.scalar_tensor_tensor(t1[:], srow["flow"][:],
                                               wf[:, m:m + 1], t1[:],
                                               op0=MUL, op1=ADD)
                rs = small.tile([P, 1], F32, tag="rs")
                nc.vector.reciprocal(rs[:], t1[:])
                ur = small.tile([P, 1], F32, tag="ur")
                uf = small.tile([P, 1], F32, tag="uf")
                nc.vector.tensor_tensor(ur[:], wr[:, m:m + 1], rs[:], op=MUL)
                nc.vector.tensor_tensor(uf[:], wf[:, m:m + 1], rs[:], op=MUL)

                # out = e_rgb*ur + e_flow*uf  (2 full-width DVE passes)
                nc.vector.tensor_scalar_mul(d["rgb"][:], d["rgb"][:], ur[:])
                nc.vector.scalar_tensor_tensor(d["rgb"][:], d["flow"][:],
                                               uf[:], d["rgb"][:],
                                               op0=MUL, op1=ADD)
                nc.sync.dma_start(out_d[m * P:(m + 1) * P, :], d["rgb"][:])

            ps_cm.__exit__(None, None, None)

    nc.compile()
    return nc


_NC_CACHE = None


def kernel(context_rgb_features, context_flow_features,
           target_rgb_features, target_flow_features, c_r, c_f):
    global _NC_CACHE
    if _NC_CACHE is None:
        _NC_CACHE = build_kernel()
    nc = _NC_CACHE

    ctx_rgb = np.asarray(context_rgb_features, np.float32)
    ctx_flow = np.asarray(context_flow_features, np.float32)
    tgt_rgb = np.asarray(target_rgb_features, np.float32)
    tgt_flow = np.asarray(target_flow_features, np.float32)
    c_r = np.asarray(c_r, np.float32)
    c_f = np.asarray(c_f, np.float32)

    ctxT_rgb = np.ascontiguousarray(ctx_rgb.T)
    ctxT_flow = np.ascontiguousarray(ctx_flow.T)

    in_maps = []
    for i in range(NCORES):
        sl = slice(i * QL, (i + 1) * QL)
        in_maps.append({
            "ctxT_rgb": ctxT_rgb,
            "ctxT_flow": ctxT_flow,
            "tgtT_rgb": np.ascontiguousarray(tgt_rgb[sl].T),
            "tgtT_flow": np.ascontiguousarray(tgt_flow[sl].T),
            "tgtR_rgb": np.ascontiguousarray(tgt_rgb[sl]),
            "tgtR_flow": np.ascontiguousarray(tgt_flow[sl]),
            "crw": np.ascontiguousarray(c_r[sl].reshape(MT, P)),
            "cfw": np.ascontiguousarray(c_f[sl].reshape(MT, P)),
        })

    global _LAST_IN_MAPS
    _LAST_IN_MAPS = in_maps
    res = run_bass_kernel_spmd(nc, in_maps, core_ids=list(range(NCORES)))
    return np.concatenate([r["out"] for r in res.results], axis=0)


_LAST_IN_MAPS = None
